# revision 21
# baseline (speedup 1.0000x reference)
"""CrossTransformerBlock (self-attn + cross-attn + MLP, post-LN) on 8 TRN2
NeuronCores.

Sharding: pure data-parallel. 8 cores = 4 batch elements x 2 sequence halves;
each core computes 512 query rows end-to-end (K/V over the full 1024-row
context are recomputed per core - no collectives).

Device layout is d-major (features on partitions, tokens on the free dim).
Matmuls run fp16 (fp32 PSUM accum); LN statistics in fp32r.

Scheduling is built around the TRN2 PE p-state ramp: the tensor engine drops
to ~1.2GHz after any idle gap and takes ~3us of continuous execution to
return to 2.4GHz, so the kernel keeps PE fed back-to-back:
 - CA K/V projections are emission-interleaved into the SA attention loops
   (and LN1) as background PE work via generator "streams".
 - softmax exp is one fused ACT over both score halves (a 2-bank PSUM tile);
   scores for step tau+1 are emitted before the AVs of step tau so PE never
   waits on the scalar engine.
 - per-head AV outputs for the odd head use a ones-FIRST V augmentation so
   they land on PSUM partitions 63..127 and the concatenated O.T needs no
   partition-shift DMAs; softmax denominators ride along as before.
 - PSUM is managed as 4 named [128,2,512] tiles (8 banks) with fixed roles
   per phase (stp double-buffer / OT accumulator / background chunks).
 - bias+residual eviction is a single scalar_tensor_tensor; LN normalize
   alternates DVE/gpsimd per d-tile; mu/rstd broadcasts are tiny PE matmuls
   into PSUM instead of gpsimd broadcasts.
 - weight slabs are 2KB-per-partition DMAs (fewer, larger transfers).
"""

import numpy as np
from collections import deque

import concourse.bass as bass
import concourse.tile as tile
from concourse import bacc, mybir
from concourse.bass_utils import run_bass_kernel_spmd

P = 128
D = 1024  # model dim
FF = 4096
H = 16  # heads
DH = 64  # head dim
S = 512  # query rows per core
T = 1024  # context rows
NC = 8  # cores
DT = D // P  # 8 d-tiles
TT = T // P  # 8 t-tiles
F32 = mybir.dt.float32
F32R = mybir.dt.float32r
F16 = mybir.dt.float16
LN_EPS = 1e-5
ADD = mybir.AluOpType.add
SUB = mybir.AluOpType.subtract
MULT = mybir.AluOpType.mult

_CACHE = {}


def _f32(ap):
    return ap.bitcast(F32)


def build():
    nc = bacc.Bacc("TRN2", target_bir_lowering=False, debug=False)

    def din(name, shape, dt=F16):
        return nc.dram_tensor(name, shape, dt, kind="ExternalInput").ap()

    xqT = din("xqT", [D, S])
    xkvT = din("xkvT", [D, T])
    memT = din("memT", [D, T])
    w = {
        name: din(name, [D, D])
        for name in ("wqsa", "wksa", "wvsa", "wosa", "wqca", "wkca", "wvca", "woca")
    }
    w1 = din("w1", [D, FF])
    w2 = din("w2", [FF, D])
    bias_dram = {
        name: din(name, [D], F32)
        for name in ("bqsa", "bksa", "bosa", "bqca", "bkca", "boca", "b2",
                     "g1", "be1", "g2", "be2", "g3", "be3")
    }
    b1_dram = din("b1", [FF], F32)
    outT = nc.dram_tensor("outT", [D, S], F32, kind="ExternalOutput").ap()

    with tile.TileContext(nc) as tc:
        _body(tc, xqT, xkvT, memT, w, w1, w2, bias_dram, b1_dram, outT)
    nc.compile()
    return nc


def _body(tc, xqT, xkvT, memT, w, w1, w2, bias_dram, b1_dram, outT):
    nc = tc.nc
    glob = tc.alloc_tile_pool(name="glob", bufs=1)
    wp = tc.alloc_tile_pool(name="wts", bufs=7)
    ps = tc.alloc_tile_pool(name="ps", bufs=1, space="PSUM")

    # 4 named PSUM tiles = 8 banks; roles rotate per phase.
    pt = [ps.tile([P, 2, S], F32, tag=f"pt{i}", name=f"pt{i}") for i in range(4)]

    _n = [0]

    def _nm(pfx):
        _n[0] += 1
        return f"{pfx}{_n[0]}"

    # ---- background PE-work streams -------------------------------------
    streams = deque()

    def feed(n=1):
        for _ in range(n):
            while streams:
                try:
                    next(streams[0])
                    break
                except StopIteration:
                    streams.popleft()

    def drain():
        while streams:
            feed(1)

    # ---- constants / params ---------------------------------------------
    def emit_bias_loads():
        bias = {}
        for name in ("bqsa", "bksa", "bosa", "bqca", "bkca", "boca", "b2",
                     "g1", "be1", "g2", "be2", "g3", "be3"):
            t = glob.tile([P, DT], F32, tag=f"c_{name}")
            nc.sync.dma_start(t[:], bias_dram[name].rearrange("(o p) -> p o", p=P))
            bias[name] = t
        b1_sb = glob.tile([P, FF // P], F32, tag="c_b1")
        nc.sync.dma_start(b1_sb[:], b1_dram.rearrange("(o p) -> p o", p=P))
        return bias, b1_sb

    ones_f32 = glob.tile([P, 1], F32, tag="ones_f32")
    nc.vector.memset(ones_f32[:], 1.0)
    ones_col = glob.tile([P, 1], F32R, tag="ones_col")
    nc.vector.tensor_copy(ones_col[:], ones_f32[:])
    ones_row = glob.tile([1, P], F32R, tag="ones_row")
    nc.vector.tensor_copy(ones_row[:], ones_f32[0:1, :].to_broadcast((1, P)))
    eps_col = glob.tile([P, 1], F32, tag="eps_col")
    nc.vector.memset(eps_col[:], LN_EPS)
    I32 = mybir.dt.int32
    magic_row = glob.tile([1, S], I32, tag="magic")
    nc.vector.memset(magic_row[:], 0x5F3759DF)

    def rsqrt_row(v_stat):
        """rstd = 1/sqrt(v) on row [0:1] via Quake seed + 3 Newton steps
        (pure DVE - keeps Ln/Exp tables off the scalar engine)."""
        y = stat_tile()
        iy = y[0:1, :].bitcast(I32)
        nc.vector.tensor_scalar(iy, v_stat[0:1, :].bitcast(I32), 1, None,
                                mybir.AluOpType.arith_shift_right)
        nc.vector.tensor_tensor(iy, magic_row[:], iy, SUB)
        t = stat_tile()
        out = y
        for it in range(3):
            nc.vector.tensor_tensor(t[0:1, :], v_stat[0:1, :], y[0:1, :], MULT)
            nc.vector.tensor_tensor(t[0:1, :], t[0:1, :], y[0:1, :], MULT)
            nc.vector.tensor_scalar(t[0:1, :], t[0:1, :], -0.5, 1.5, MULT, ADD)
            if it == 2:
                out = stat_r_tile()
                nc.vector.tensor_tensor(out[:], y[0:1, :], t[0:1, :], MULT)
            else:
                nc.vector.tensor_tensor(y[0:1, :], y[0:1, :], t[0:1, :], MULT)
        return out

    def stat_tile():
        return glob.tile([65, S], F32, tag="stat", bufs=4, name=_nm("stat"))

    def stat_r_tile():  # fp32r rows usable as fp32r-matmul rhs
        return glob.tile([1, S], F32R, tag="statr", bufs=2, name=_nm("str"))

    def bc_tile():  # per-head reciprocal denominators, broadcast on rows 0:64
        return glob.tile([64, 2, S], F32, tag="bc", bufs=2, name=_nm("bc"))

    def avstg_tile():
        return glob.tile([64, S], F16, tag="avstg", bufs=2, name=_nm("avstg"))

    def nrm_tile():
        return glob.tile([P, S], F32, tag="nrm", bufs=2, name=_nm("nrm"))

    def sq_tile():
        return glob.tile([P, S], F32R, tag="sq", bufs=2, name=_nm("sq"))

    def r_tile():  # pre-LN residual sums (fp32r so LN stats keep precision)
        return glob.tile([P, DT, S], F32R, tag="r", bufs=1, name=_nm("r"))

    def lnout_tile():  # x1T / x2T
        return glob.tile([P, DT, S], F16, tag="lnout", bufs=2, name=_nm("lnout"))

    def wslab():  # generic 2KB/partition weight slab
        return wp.tile([P, 1024], F16, tag="wslab", name=_nm("w"))

    def vslab():
        return wp.tile([P, 2, S], F16, tag="vslab", bufs=4, name=_nm("vw"))

    # ---- helpers ---------------------------------------------------------
    def kproj_stream(wd, src, bcol, KT, g, accs=(3,)):
        """K projection for head group g: KT[:, jj, :] = (wd cols).T @ src,
        t-major K.T, one dtile (=2 heads) per chunk."""
        for jj in range(4):
            acc = pt[accs[jj % len(accs)]]
            dt_i = g * 4 + jj
            slab = wslab()
            sl = slab[:].rearrange("p (k c) -> p k c", c=P)  # [P, 8, 128]
            nc.sync.dma_start(
                sl[:],
                wd[:, dt_i * P:(dt_i + 1) * P].rearrange("(k p) c -> p k c", p=P),
            )
            for k in range(DT):
                for ht in range(2):
                    nc.tensor.matmul(
                        acc[:, ht, :], sl[:, k, :], src[:, k, ht * S:(ht + 1) * S],
                        start=(k == 0), stop=(k == DT - 1),
                    )
                yield
            nc.vector.tensor_scalar_add(
                KT[:, jj, :], acc[:].rearrange("p two s -> p (two s)"),
                bcol[:, dt_i:dt_i + 1],
            )
            yield

    def vproj_stream(wd, src, g, Vg, accs=(3,)):
        """V projection for head group g, t-major, augmented with a ones
        column per head (AV then emits softmax denominators on psum row 64).
        Two taus per chunk."""
        sls = []
        for i in range(4):
            vs = vslab()
            nc.sync.dma_start(
                vs[:],
                wd[2 * i * P:(2 * i + 2) * P, g * S:(g + 1) * S]
                .rearrange("(kk p) c -> p kk c", p=P),
            )
            sls.append(vs)
        nc.vector.tensor_copy(
            Vg[:, :, :, DH:DH + 1], ones_f32[:].to_broadcast((P, TT, 8, 1)))
        for tc2 in range(4):
            acc = pt[accs[tc2 % len(accs)]]
            for k in range(DT):
                vs = sls[k // 2]
                for tl in range(2):
                    tau = tc2 * 2 + tl
                    nc.tensor.matmul(
                        acc[:, tl, :], src[:, k, tau * P:(tau + 1) * P],
                        vs[:, k % 2, :],
                        start=(k == 0), stop=(k == DT - 1),
                    )
                yield
            for tl in range(2):
                tau = tc2 * 2 + tl
                nc.vector.tensor_copy(
                    Vg[:, tau, :, 0:DH],
                    acc[:, tl, :].rearrange("p (h d) -> p h d", h=8))
            yield

    def proj2(dst, wd, rhs_fn, bcol, resid_fn=None, pts=(0, 1, 2), dst_f32r=False):
        """dst[:, o, :] = W-slab.T @ rhs (+bias, +residual); 4 chunks of 2
        output dtiles rotating over pt[pts]."""
        for c in range(4):
            acc = pt[pts[c % len(pts)]]
            for half in range(2):
                slab = wslab()
                sl = slab[:].rearrange("p (kk c) -> p kk c", c=2 * P)  # [P,4,256]
                nc.sync.dma_start(
                    sl[:],
                    wd[half * 4 * P:(half + 1) * 4 * P, c * 2 * P:(c + 1) * 2 * P]
                    .rearrange("(kk p) c -> p kk c", p=P),
                )
                for kk in range(4):
                    k = half * 4 + kk
                    for o2 in range(2):
                        nc.tensor.matmul(
                            acc[:, o2, :], sl[:, kk, o2 * P:(o2 + 1) * P],
                            rhs_fn(k),
                            start=(k == 0), stop=(k == DT - 1),
                        )
                feed(1)
            for o2 in range(2):
                o = c * 2 + o2
                d = dst[:, o, :]
                if resid_fn is not None:
                    nc.vector.scalar_tensor_tensor(
                        d, acc[:, o2, :], bcol[:, o:o + 1], resid_fn(o), ADD, ADD)
                else:
                    nc.vector.tensor_scalar_add(d, acc[:, o2, :], bcol[:, o:o + 1])
            feed(1)

    def layernorm(r, g_col, b_col, dst, stat_pt, per_k_done=None):
        """dst[:, k, :] = LN(r) over d. Stats (PE ones-matmuls) accumulate in
        stat_pt rows [0:1]; mu/rstd broadcast back into the same two banks by
        tiny PE matmuls. Normalize alternates DVE / gpsimd per k."""
        sum_ap = stat_pt[0:1, 0, :]
        sq_ap = stat_pt[0:1, 1, :]
        for k in range(DT):
            sq = sq_tile()
            nc.vector.tensor_tensor(
                sq[:], _f32(r[:, k, :]), _f32(r[:, k, :]), MULT)
            nc.tensor.matmul(sum_ap, ones_col[:], r[:, k, :],
                             start=(k == 0), stop=(k == DT - 1))
            nc.tensor.matmul(sq_ap, ones_col[:], sq[:],
                             start=(k == 0), stop=(k == DT - 1))
            feed(1)
        mu = stat_r_tile()
        nc.vector.tensor_scalar_mul(mu[:], sum_ap, 1.0 / D)
        var = stat_tile()
        nc.vector.tensor_scalar_mul(var[0:1, :], sq_ap, 1.0 / D)
        aux = stat_tile()
        nc.vector.tensor_tensor(aux[0:1, :], _f32(mu[:]), _f32(mu[:]), MULT)
        nc.vector.scalar_tensor_tensor(var[0:1, :], var[0:1, :],
                                       eps_col[0:1, :], aux[0:1, :], ADD, SUB)
        rstd = rsqrt_row(var)
        # broadcast mu/rstd across partitions via PE into the stats banks
        nc.tensor.matmul(stat_pt[:, 0, :], ones_row[:], mu[:],
                         start=True, stop=True)
        nc.tensor.matmul(stat_pt[:, 1, :], ones_row[:], rstd[:],
                         start=True, stop=True)
        for k in range(DT):
            t1 = nrm_tile()
            nc.vector.tensor_tensor(t1[:], _f32(r[:, k, :]), stat_pt[:, 0, :], SUB)
            nc.vector.tensor_tensor(t1[:], t1[:], stat_pt[:, 1, :], MULT)
            eng = nc.gpsimd if k % 2 else nc.vector
            eng.tensor_scalar(
                dst[:, k, :], t1[:], g_col[:, k:k + 1], b_col[:, k:k + 1],
                MULT, ADD)
            if per_k_done is not None:
                per_k_done(k)
            feed(1)

    # =================== attention phase ==================================
    with tc.tile_pool(name="attn", bufs=1) as at:
        srcx = at.tile([P, DT, T], F16, tag="srcx")
        for k in range(DT):
            nc.sync.dma_start(srcx[:, k, :], xkvT[k * P:(k + 1) * P, :])
        xq = at.tile([P, DT, S], F16, tag="xq")
        nc.sync.dma_start(xq[:], xqT.rearrange("(k p) s -> p k s", p=P))
        bias, b1_sb = emit_bias_loads()
        srcm = at.tile([P, DT, T], F16, tag="srcm")
        for k in range(DT):
            nc.sync.dma_start(srcm[:, k, :], memT[k * P:(k + 1) * P, :])

        def KT_tile():
            return at.tile([P, 4, T], F16, tag="at_KT", bufs=3, name=_nm("KT"))

        def Vg_tile():
            return at.tile([P, TT, 8, DH + 1], F16, tag="at_V", bufs=3,
                           name=_nm("V"))

        def est_tile():
            return at.tile([P, 2, S], F16, tag="est", bufs=2, name=_nm("est"))

        QT = at.tile([P, DT, S], F16, tag="at_QT", bufs=1, name="QT")

        def OT_tile():
            return at.tile([P, DT, S], F16, tag="at_OT", bufs=2, name=_nm("OT"))

        def att_group(g, KTg, Vgg, QTt, OT, ot_list=(2,)):
            """scores -> fused exp -> AV for head group g. Scores for step
            tau are emitted before the AVs of step tau-1 so PE stays ahead
            of the scalar engine; feed() interleaves background chunks. The
            AV accumulator alternates over ot_list so the eviction chain of
            head-pair j overlaps head-pair j+1."""
            stp_i = [0]

            def av(tau, est, j, ot):
                for half in range(2):
                    nc.tensor.matmul(ot[0:65, half, :],
                                     Vgg[:, tau, 2 * j + half, :],
                                     est[:, half, :],
                                     start=(tau == 0), stop=(tau == TT - 1))

            for j in range(4):
                ot = pt[ot_list[j % len(ot_list)]]
                dtile = g * 4 + j
                prev = None
                for tau in range(TT):
                    stp = pt[stp_i[0]]
                    stp_i[0] ^= 1
                    for half in range(2):
                        r0 = 64 * half
                        nc.tensor.matmul(
                            stp[:, half, :],
                            KTg[r0:r0 + 64, j, tau * P:(tau + 1) * P],
                            QTt[r0:r0 + 64, dtile, :],
                            start=True, stop=True, tile_position=(r0, 0),
                        )
                    feed(2)
                    if prev is not None:
                        av(prev[0], prev[1], j, ot)
                    est = est_tile()
                    nc.scalar.activation(
                        est[:], stp[:], mybir.ActivationFunctionType.Exp,
                        scale=0.125)
                    prev = (tau, est)
                av(prev[0], prev[1], j, ot)
                # normalize: psum row 64 of each half is the softmax
                # denominator (V-aug ones column).
                bch = bc_tile()
                for half in range(2):
                    st = stat_tile()
                    nc.vector.tensor_copy(st[64:65, :], ot[64:65, half, :])
                    sh = stat_tile()
                    nc.scalar.dma_start(sh[0:1, :], st[64:65, :])
                    rc = stat_tile()
                    nc.vector.reciprocal_approx_fast(rc[0:1, :], sh[0:1, :])
                    nc.gpsimd.partition_broadcast(bch[:, half, :], rc[0:1, :])
                nc.vector.tensor_tensor(
                    OT[0:64, dtile, :], ot[0:64, 0, :], bch[:, 0, :], MULT)
                stg = avstg_tile()
                nc.vector.tensor_tensor(
                    stg[:], ot[0:64, 1, :], bch[:, 1, :], MULT)
                nc.sync.dma_start(OT[64:128, dtile, :], stg[:])
                feed(6)

        # ---- SA prep (PE fully busy, no dependencies) --------------------
        KT_sa = [KT_tile(), KT_tile()]
        Vg_sa = [Vg_tile(), Vg_tile()]
        streams.append(kproj_stream(w["wksa"], srcx, bias["bksa"], KT_sa[0], 0, accs=(0, 1, 2, 3)))
        drain()
        streams.append(vproj_stream(w["wvsa"], srcx, 0, Vg_sa[0], accs=(0, 1, 2, 3)))
        drain()
        proj2(QT, w["wqsa"], lambda k: xq[:, k, :], bias["bqsa"])
        streams.append(kproj_stream(w["wksa"], srcx, bias["bksa"], KT_sa[1], 1, accs=(0, 1, 2, 3)))
        drain()
        streams.append(vproj_stream(w["wvsa"], srcx, 1, Vg_sa[1], accs=(0, 1, 2, 3)))
        drain()

        # ---- SA attention; CA K/V projections ride along as background ---
        OT1 = OT_tile()
        KT_ca = [KT_tile(), KT_tile()]
        Vg_ca = [Vg_tile(), Vg_tile()]
        streams.append(kproj_stream(w["wkca"], srcm, bias["bkca"], KT_ca[0], 0))
        streams.append(vproj_stream(w["wvca"], srcm, 0, Vg_ca[0]))
        att_group(0, KT_sa[0], Vg_sa[0], QT, OT1)
        streams.append(kproj_stream(w["wkca"], srcm, bias["bkca"], KT_ca[1], 1))
        att_group(1, KT_sa[1], Vg_sa[1], QT, OT1)

        streams.append(vproj_stream(w["wvca"], srcm, 1, Vg_ca[1]))
        r1 = r_tile()
        proj2(r1, w["wosa"], lambda k: OT1[:, k, :], bias["bosa"],
              resid_fn=lambda o: xq[:, o, :], pts=(0, 1), dst_f32r=True)
        x1T = lnout_tile()
        layernorm(r1, bias["g1"], bias["be1"], x1T, pt[2])

        # ---- CA ----------------------------------------------------------
        proj2(QT, w["wqca"], lambda k: x1T[:, k, :], bias["bqca"])
        OT2 = OT_tile()
        att_group(0, KT_ca[0], Vg_ca[0], QT, OT2, ot_list=(2, 3))
        att_group(1, KT_ca[1], Vg_ca[1], QT, OT2, ot_list=(2, 3))
        drain()

        r2 = r_tile()
        proj2(r2, w["woca"], lambda k: OT2[:, k, :], bias["boca"],
              resid_fn=lambda o: x1T[:, o, :], pts=(0, 1), dst_f32r=True)
        x2T = lnout_tile()
        layernorm(r2, bias["g2"], bias["be2"], x2T, pt[3])

    # ======================= MLP phase ===================================
    with tc.tile_pool(name="mlp", bufs=1) as mlp:
        hT = mlp.tile([P, FF // P, S], F16, tag="hT")
        for c in range(16):
            acc = pt[c % 4]
            for half in range(2):
                slab = wslab()
                sl = slab[:].rearrange("p (kk c) -> p kk c", c=2 * P)
                nc.sync.dma_start(
                    sl[:],
                    w1[half * 4 * P:(half + 1) * 4 * P,
                       c * 2 * P:(c + 1) * 2 * P]
                    .rearrange("(kk p) c -> p kk c", p=P),
                )
                for kk in range(4):
                    k = half * 4 + kk
                    for f2 in range(2):
                        nc.tensor.matmul(
                            acc[:, f2, :], sl[:, kk, f2 * P:(f2 + 1) * P],
                            x2T[:, k, :],
                            start=(k == 0), stop=(k == DT - 1),
                        )
            for f2 in range(2):
                fi = c * 2 + f2
                nc.scalar.activation(
                    hT[:, fi, :], acc[:, f2, :],
                    mybir.ActivationFunctionType.Gelu,
                    bias=b1_sb[:, fi:fi + 1])

        r3 = r_tile()
        sum_ap = pt[3][0:1, 0, :]
        sq_ap = pt[3][0:1, 1, :]
        for c in range(4):
            acc = pt[c % 3]
            for part in range(8):
                slab = wslab()
                sl = slab[:].rearrange("p (kk c) -> p kk c", c=2 * P)
                nc.sync.dma_start(
                    sl[:],
                    w2[part * 4 * P:(part + 1) * 4 * P,
                       c * 2 * P:(c + 1) * 2 * P]
                    .rearrange("(kk p) c -> p kk c", p=P),
                )
                for kk in range(4):
                    f = part * 4 + kk
                    for o2 in range(2):
                        nc.tensor.matmul(
                            acc[:, o2, :], sl[:, kk, o2 * P:(o2 + 1) * P],
                            hT[:, f, :],
                            start=(f == 0), stop=(f == FF // P - 1),
                        )
            for o2 in range(2):
                o = c * 2 + o2
                nc.vector.scalar_tensor_tensor(
                    r3[:, o, :], acc[:, o2, :], bias["b2"][:, o:o + 1],
                    x2T[:, o, :], ADD, ADD)
                sq = sq_tile()
                nc.vector.tensor_tensor(
                    sq[:], _f32(r3[:, o, :]), _f32(r3[:, o, :]), MULT)
                nc.tensor.matmul(sum_ap, ones_col[:], r3[:, o, :],
                                 start=(o == 0), stop=(o == DT - 1))
                nc.tensor.matmul(sq_ap, ones_col[:], sq[:],
                                 start=(o == 0), stop=(o == DT - 1))

        # LN3 (stats already accumulated in pt[3]) + per-k output DMA
        mu = stat_r_tile()
        nc.vector.tensor_scalar_mul(mu[:], sum_ap, 1.0 / D)
        var = stat_tile()
        nc.vector.tensor_scalar_mul(var[0:1, :], sq_ap, 1.0 / D)
        aux = stat_tile()
        nc.vector.tensor_tensor(aux[0:1, :], _f32(mu[:]), _f32(mu[:]), MULT)
        nc.vector.scalar_tensor_tensor(var[0:1, :], var[0:1, :],
                                       eps_col[0:1, :], aux[0:1, :], ADD, SUB)
        rstd = rsqrt_row(var)
        nc.tensor.matmul(pt[3][:, 0, :], ones_row[:], mu[:],
                         start=True, stop=True)
        nc.tensor.matmul(pt[3][:, 1, :], ones_row[:], rstd[:],
                         start=True, stop=True)
        for k in range(DT):
            t1 = nrm_tile()
            nc.vector.tensor_tensor(t1[:], _f32(r3[:, k, :]), pt[3][:, 0, :], SUB)
            nc.vector.tensor_tensor(t1[:], t1[:], pt[3][:, 1, :], MULT)
            outsb = glob.tile([P, S], F32, tag="outsb", bufs=2, name=_nm("ou"))
            eng = nc.gpsimd if k % 2 else nc.vector
            eng.tensor_scalar(
                outsb[:], t1[:], bias["g3"][:, k:k + 1], bias["be3"][:, k:k + 1],
                MULT, ADD)
            nc.sync.dma_start(outT[k * P:(k + 1) * P, :], outsb[:])

    ps.release()
    wp.release()
    glob.release()


def _get_nc():
    if "nc" not in _CACHE:
        _CACHE["nc"] = build()
    return _CACHE["nc"]


def kernel(x, mem, sa_in_w, sa_in_b, sa_out_w, sa_out_b,
           ca_in_w, ca_in_b, ca_out_w, ca_out_b,
           ff_w1, ff_b1, ff_w2, ff_b2,
           ln1_g, ln1_b, ln2_g, ln2_b, ln3_g, ln3_b, n_heads=16):
    x = np.asarray(x, np.float32)
    mem = np.asarray(mem, np.float32)
    B = x.shape[0]

    def T_(a):
        return np.ascontiguousarray(np.asarray(a, np.float32).T.astype(np.float16))

    wq_sa, wk_sa, wv_sa = (np.asarray(a, np.float32) for a in np.split(np.asarray(sa_in_w), 3, axis=0))
    bq_sa, bk_sa, bv_sa = (np.asarray(a, np.float32) for a in np.split(np.asarray(sa_in_b), 3))
    wq_ca, wk_ca, wv_ca = (np.asarray(a, np.float32) for a in np.split(np.asarray(ca_in_w), 3, axis=0))
    bq_ca, bk_ca, bv_ca = (np.asarray(a, np.float32) for a in np.split(np.asarray(ca_in_b), 3))
    sa_out_w = np.asarray(sa_out_w, np.float32)
    ca_out_w = np.asarray(ca_out_w, np.float32)

    common = {
        "wqsa": T_(wq_sa), "wksa": T_(wk_sa), "wvsa": T_(wv_sa), "wosa": T_(sa_out_w),
        "wqca": T_(wq_ca), "wkca": T_(wk_ca), "wvca": T_(wv_ca), "woca": T_(ca_out_w),
        "w1": T_(ff_w1), "w2": T_(ff_w2),
        "bqsa": bq_sa, "bksa": bk_sa,
        "bosa": np.asarray(sa_out_b, np.float32) + sa_out_w @ bv_sa,
        "bqca": bq_ca, "bkca": bk_ca,
        "boca": np.asarray(ca_out_b, np.float32) + ca_out_w @ bv_ca,
        "b1": np.asarray(ff_b1, np.float32), "b2": np.asarray(ff_b2, np.float32),
        "g1": np.asarray(ln1_g, np.float32), "be1": np.asarray(ln1_b, np.float32),
        "g2": np.asarray(ln2_g, np.float32), "be2": np.asarray(ln2_b, np.float32),
        "g3": np.asarray(ln3_g, np.float32), "be3": np.asarray(ln3_b, np.float32),
    }

    in_maps = []
    for c in range(NC):
        b, h = c // 2, c % 2
        xbT = T_(x[b])
        in_maps.append({
            **common,
            "xqT": np.ascontiguousarray(xbT[:, h * S: (h + 1) * S]),
            "xkvT": xbT,
            "memT": T_(mem[b]),
        })

    nc = _get_nc()
    res = run_bass_kernel_spmd(nc, in_maps, core_ids=list(range(NC)))

    out = np.empty((B, T, D), np.float32)
    for c in range(NC):
        b, h = c // 2, c % 2
        out[b, h * S: (h + 1) * S, :] = res.results[c]["outT"].T
    return out


# revision 22
# speedup vs baseline: 1.0093x; 1.0093x over previous
"""CrossTransformerBlock (self-attn + cross-attn + MLP, post-LN) on 8 TRN2
NeuronCores.

Sharding: pure data-parallel. 8 cores = 4 batch elements x 2 sequence halves;
each core computes 512 query rows end-to-end (K/V over the full 1024-row
context are recomputed per core - no collectives).

Device layout is d-major (features on partitions, tokens on the free dim).
Matmuls run fp16 (fp32 PSUM accum); LN statistics in fp32r.

Scheduling is built around the TRN2 PE p-state ramp: the tensor engine drops
to ~1.2GHz after any idle gap and takes ~3us of continuous execution to
return to 2.4GHz, so the kernel keeps PE fed back-to-back:
 - CA K/V projections are emission-interleaved into the SA attention loops
   (and LN1) as background PE work via generator "streams".
 - softmax exp is one fused ACT over both score halves (a 2-bank PSUM tile);
   scores for step tau+1 are emitted before the AVs of step tau so PE never
   waits on the scalar engine.
 - per-head AV outputs for the odd head use a ones-FIRST V augmentation so
   they land on PSUM partitions 63..127 and the concatenated O.T needs no
   partition-shift DMAs; softmax denominators ride along as before.
 - PSUM is managed as 4 named [128,2,512] tiles (8 banks) with fixed roles
   per phase (stp double-buffer / OT accumulator / background chunks).
 - bias+residual eviction is a single scalar_tensor_tensor; LN normalize
   alternates DVE/gpsimd per d-tile; mu/rstd broadcasts are tiny PE matmuls
   into PSUM instead of gpsimd broadcasts.
 - weight slabs are 2KB-per-partition DMAs (fewer, larger transfers).
"""

import numpy as np
from collections import deque

import concourse.bass as bass
import concourse.tile as tile
from concourse import bacc, mybir
from concourse.bass_utils import run_bass_kernel_spmd

P = 128
D = 1024  # model dim
FF = 4096
H = 16  # heads
DH = 64  # head dim
S = 512  # query rows per core
T = 1024  # context rows
NC = 8  # cores
DT = D // P  # 8 d-tiles
TT = T // P  # 8 t-tiles
F32 = mybir.dt.float32
F32R = mybir.dt.float32r
F16 = mybir.dt.float16
LN_EPS = 1e-5
ADD = mybir.AluOpType.add
SUB = mybir.AluOpType.subtract
MULT = mybir.AluOpType.mult

_CACHE = {}


def _f32(ap):
    return ap.bitcast(F32)


def build():
    nc = bacc.Bacc("TRN2", target_bir_lowering=False, debug=False)

    def din(name, shape, dt=F16):
        return nc.dram_tensor(name, shape, dt, kind="ExternalInput").ap()

    xqT = din("xqT", [D, S])
    xkvT = din("xkvT", [D, T])
    memT = din("memT", [D, T])
    w = {
        name: din(name, [D, D])
        for name in ("wqsa", "wksa", "wvsa", "wosa", "wqca", "wkca", "wvca", "woca")
    }
    w1 = din("w1", [D, FF])
    w2 = din("w2", [FF, D])
    bias_dram = {
        name: din(name, [D], F32)
        for name in ("bqsa", "bksa", "bosa", "bqca", "bkca", "boca", "b2",
                     "g1", "be1", "g2", "be2", "g3", "be3")
    }
    b1_dram = din("b1", [FF], F32)
    outT = nc.dram_tensor("outT", [D, S], F32, kind="ExternalOutput").ap()

    with tile.TileContext(nc) as tc:
        _body(tc, xqT, xkvT, memT, w, w1, w2, bias_dram, b1_dram, outT)
    nc.compile()
    return nc


def _body(tc, xqT, xkvT, memT, w, w1, w2, bias_dram, b1_dram, outT):
    nc = tc.nc
    glob = tc.alloc_tile_pool(name="glob", bufs=1)
    wp = tc.alloc_tile_pool(name="wts", bufs=7)
    ps = tc.alloc_tile_pool(name="ps", bufs=1, space="PSUM")

    # 4 named PSUM tiles = 8 banks; roles rotate per phase.
    pt = [ps.tile([P, 2, S], F32, tag=f"pt{i}", name=f"pt{i}") for i in range(4)]

    _n = [0]

    def _nm(pfx):
        _n[0] += 1
        return f"{pfx}{_n[0]}"

    # ---- background PE-work streams -------------------------------------
    streams = deque()

    def feed(n=1):
        for _ in range(n):
            while streams:
                try:
                    next(streams[0])
                    break
                except StopIteration:
                    streams.popleft()

    def drain():
        while streams:
            feed(1)

    # ---- constants / params ---------------------------------------------
    def emit_bias_loads():
        bias = {}
        for name in ("bqsa", "bksa", "bosa", "bqca", "bkca", "boca", "b2",
                     "g1", "be1", "g2", "be2", "g3", "be3"):
            t = glob.tile([P, DT], F32, tag=f"c_{name}")
            nc.sync.dma_start(t[:], bias_dram[name].rearrange("(o p) -> p o", p=P))
            bias[name] = t
        b1_sb = glob.tile([P, FF // P], F32, tag="c_b1")
        nc.sync.dma_start(b1_sb[:], b1_dram.rearrange("(o p) -> p o", p=P))
        return bias, b1_sb

    ones_f32 = glob.tile([P, 1], F32, tag="ones_f32")
    nc.vector.memset(ones_f32[:], 1.0)
    ones_col = glob.tile([P, 1], F32R, tag="ones_col")
    nc.vector.tensor_copy(ones_col[:], ones_f32[:])
    ones_row = glob.tile([1, P], F32R, tag="ones_row")
    nc.vector.tensor_copy(ones_row[:], ones_f32[0:1, :].to_broadcast((1, P)))
    eps_col = glob.tile([P, 1], F32, tag="eps_col")
    nc.vector.memset(eps_col[:], LN_EPS)
    I32 = mybir.dt.int32
    magic_row = glob.tile([1, S], I32, tag="magic")
    nc.vector.memset(magic_row[:], 0x5F3759DF)

    def rsqrt_row(v_stat):
        """rstd = 1/sqrt(v) on row [0:1] via Quake seed + 3 Newton steps
        (pure DVE - keeps Ln/Exp tables off the scalar engine)."""
        y = stat_tile()
        iy = y[0:1, :].bitcast(I32)
        nc.vector.tensor_scalar(iy, v_stat[0:1, :].bitcast(I32), 1, None,
                                mybir.AluOpType.arith_shift_right)
        nc.vector.tensor_tensor(iy, magic_row[:], iy, SUB)
        t = stat_tile()
        out = y
        for it in range(2):
            feed(1)
            nc.vector.tensor_tensor(t[0:1, :], v_stat[0:1, :], y[0:1, :], MULT)
            nc.vector.tensor_tensor(t[0:1, :], t[0:1, :], y[0:1, :], MULT)
            nc.vector.tensor_scalar(t[0:1, :], t[0:1, :], -0.5, 1.5, MULT, ADD)
            if it == 1:
                out = stat_r_tile()
                nc.vector.tensor_tensor(out[:], y[0:1, :], t[0:1, :], MULT)
            else:
                nc.vector.tensor_tensor(y[0:1, :], y[0:1, :], t[0:1, :], MULT)
        return out

    def stat_tile():
        return glob.tile([65, S], F32, tag="stat", bufs=4, name=_nm("stat"))

    def stat_r_tile():  # fp32r rows usable as fp32r-matmul rhs
        return glob.tile([1, S], F32R, tag="statr", bufs=2, name=_nm("str"))

    def bc_tile():  # per-head reciprocal denominators, broadcast on rows 0:64
        return glob.tile([64, 2, S], F32, tag="bc", bufs=2, name=_nm("bc"))

    def avstg_tile():
        return glob.tile([64, S], F16, tag="avstg", bufs=2, name=_nm("avstg"))

    def nrm_tile():
        return glob.tile([P, S], F32, tag="nrm", bufs=2, name=_nm("nrm"))

    def sq_tile():
        return glob.tile([P, S], F32R, tag="sq", bufs=2, name=_nm("sq"))

    def r_tile():  # pre-LN residual sums (fp32r so LN stats keep precision)
        return glob.tile([P, DT, S], F32R, tag="r", bufs=1, name=_nm("r"))

    def lnout_tile():  # x1T / x2T
        return glob.tile([P, DT, S], F16, tag="lnout", bufs=2, name=_nm("lnout"))

    def wslab():  # generic 2KB/partition weight slab
        return wp.tile([P, 1024], F16, tag="wslab", name=_nm("w"))

    def vslab():
        return wp.tile([P, 2, S], F16, tag="vslab", bufs=4, name=_nm("vw"))

    # ---- helpers ---------------------------------------------------------
    def kproj_stream(wd, src, bcol, KT, g, accs=(3,)):
        """K projection for head group g: KT[:, jj, :] = (wd cols).T @ src,
        t-major K.T, one dtile (=2 heads) per chunk."""
        for jj in range(4):
            acc = pt[accs[jj % len(accs)]]
            dt_i = g * 4 + jj
            slab = wslab()
            sl = slab[:].rearrange("p (k c) -> p k c", c=P)  # [P, 8, 128]
            nc.sync.dma_start(
                sl[:],
                wd[:, dt_i * P:(dt_i + 1) * P].rearrange("(k p) c -> p k c", p=P),
            )
            for k in range(DT):
                for ht in range(2):
                    nc.tensor.matmul(
                        acc[:, ht, :], sl[:, k, :], src[:, k, ht * S:(ht + 1) * S],
                        start=(k == 0), stop=(k == DT - 1),
                    )
                yield
            nc.vector.tensor_scalar_add(
                KT[:, jj, :], acc[:].rearrange("p two s -> p (two s)"),
                bcol[:, dt_i:dt_i + 1],
            )
            yield

    def vproj_stream(wd, src, g, Vg, accs=(3,)):
        """V projection for head group g, t-major, augmented with a ones
        column per head (AV then emits softmax denominators on psum row 64).
        Two taus per chunk."""
        sls = []
        for i in range(4):
            vs = vslab()
            nc.sync.dma_start(
                vs[:],
                wd[2 * i * P:(2 * i + 2) * P, g * S:(g + 1) * S]
                .rearrange("(kk p) c -> p kk c", p=P),
            )
            sls.append(vs)
        nc.vector.tensor_copy(
            Vg[:, :, :, DH:DH + 1], ones_f32[:].to_broadcast((P, TT, 8, 1)))
        for tc2 in range(4):
            acc = pt[accs[tc2 % len(accs)]]
            for k in range(DT):
                vs = sls[k // 2]
                for tl in range(2):
                    tau = tc2 * 2 + tl
                    nc.tensor.matmul(
                        acc[:, tl, :], src[:, k, tau * P:(tau + 1) * P],
                        vs[:, k % 2, :],
                        start=(k == 0), stop=(k == DT - 1),
                    )
                yield
            for tl in range(2):
                tau = tc2 * 2 + tl
                nc.vector.tensor_copy(
                    Vg[:, tau, :, 0:DH],
                    acc[:, tl, :].rearrange("p (h d) -> p h d", h=8))
            yield

    def proj2(dst, wd, rhs_fn, bcol, resid_fn=None, pts=(0, 1, 2),
              per_chunk=None):
        """dst[:, o, :] = W-slab.T @ rhs (+bias, +residual); 4 chunks of 2
        output dtiles rotating over pt[pts]. per_chunk(c) runs after chunk
        c's eviction (used to interleave LN stats with the out-proj)."""
        for c in range(4):
            acc = pt[pts[c % len(pts)]]
            for half in range(2):
                slab = wslab()
                sl = slab[:].rearrange("p (kk c) -> p kk c", c=2 * P)  # [P,4,256]
                nc.sync.dma_start(
                    sl[:],
                    wd[half * 4 * P:(half + 1) * 4 * P, c * 2 * P:(c + 1) * 2 * P]
                    .rearrange("(kk p) c -> p kk c", p=P),
                )
                for kk in range(4):
                    k = half * 4 + kk
                    for o2 in range(2):
                        nc.tensor.matmul(
                            acc[:, o2, :], sl[:, kk, o2 * P:(o2 + 1) * P],
                            rhs_fn(k),
                            start=(k == 0), stop=(k == DT - 1),
                        )
                    if kk % 2 == 1:
                        feed(1)
            for o2 in range(2):
                o = c * 2 + o2
                d = dst[:, o, :]
                if resid_fn is not None:
                    nc.vector.scalar_tensor_tensor(
                        d, acc[:, o2, :], bcol[:, o:o + 1], resid_fn(o), ADD, ADD)
                else:
                    nc.vector.tensor_scalar_add(d, acc[:, o2, :], bcol[:, o:o + 1])
            if per_chunk is not None:
                per_chunk(c)
            feed(1)

    def ln_stats(r, stat_pt, ks):
        """accumulate sum / sum-of-squares rows for r[:, k, :] (PE
        ones-matmuls into stat_pt rows [0:1])."""
        for k in ks:
            sq = sq_tile()
            nc.vector.tensor_tensor(
                sq[:], _f32(r[:, k, :]), _f32(r[:, k, :]), MULT)
            nc.tensor.matmul(stat_pt[0:1, 0, :], ones_col[:], r[:, k, :],
                             start=(k == 0), stop=(k == DT - 1))
            nc.tensor.matmul(stat_pt[0:1, 1, :], ones_col[:], sq[:],
                             start=(k == 0), stop=(k == DT - 1))

    def layernorm(r, g_col, b_col, dst, stat_pt, stats_done=False):
        """dst[:, k, :] = LN(r) over d. Stats accumulate in stat_pt rows
        [0:1]; mu/rstd broadcast back into the same two banks by tiny PE
        matmuls. Normalize alternates DVE / gpsimd per k."""
        sum_ap = stat_pt[0:1, 0, :]
        sq_ap = stat_pt[0:1, 1, :]
        if not stats_done:
            for k in range(DT):
                ln_stats(r, stat_pt, [k])
                feed(1)
        mu = stat_r_tile()
        nc.vector.tensor_scalar_mul(mu[:], sum_ap, 1.0 / D)
        var = stat_tile()
        nc.vector.tensor_scalar_mul(var[0:1, :], sq_ap, 1.0 / D)
        feed(1)
        aux = stat_tile()
        nc.vector.tensor_tensor(aux[0:1, :], _f32(mu[:]), _f32(mu[:]), MULT)
        nc.vector.scalar_tensor_tensor(var[0:1, :], var[0:1, :],
                                       eps_col[0:1, :], aux[0:1, :], ADD, SUB)
        feed(1)
        rstd = rsqrt_row(var)
        feed(1)
        # broadcast mu/rstd across partitions via PE into the stats banks
        nc.tensor.matmul(stat_pt[:, 0, :], ones_row[:], mu[:],
                         start=True, stop=True)
        nc.tensor.matmul(stat_pt[:, 1, :], ones_row[:], rstd[:],
                         start=True, stop=True)
        for k in range(DT):
            t1 = nrm_tile()
            nc.vector.tensor_tensor(t1[:], _f32(r[:, k, :]), stat_pt[:, 0, :], SUB)
            nc.vector.tensor_tensor(t1[:], t1[:], stat_pt[:, 1, :], MULT)
            eng = nc.gpsimd if k % 2 else nc.vector
            eng.tensor_scalar(
                dst[:, k, :], t1[:], g_col[:, k:k + 1], b_col[:, k:k + 1],
                MULT, ADD)
            feed(1)

    # =================== attention phase ==================================
    with tc.tile_pool(name="attn", bufs=1) as at:
        srcx = at.tile([P, DT, T], F16, tag="srcx")
        for k in range(DT):
            nc.sync.dma_start(srcx[:, k, :], xkvT[k * P:(k + 1) * P, :])
        xq = at.tile([P, DT, S], F16, tag="xq")
        nc.sync.dma_start(xq[:], xqT.rearrange("(k p) s -> p k s", p=P))
        bias, b1_sb = emit_bias_loads()
        srcm = at.tile([P, DT, T], F16, tag="srcm")
        for k in range(DT):
            nc.sync.dma_start(srcm[:, k, :], memT[k * P:(k + 1) * P, :])

        def KT_tile():
            return at.tile([P, 4, T], F16, tag="at_KT", bufs=3, name=_nm("KT"))

        def Vg_tile():
            return at.tile([P, TT, 8, DH + 1], F16, tag="at_V", bufs=3,
                           name=_nm("V"))

        def est_tile():
            return at.tile([P, 2, S], F16, tag="est", bufs=2, name=_nm("est"))

        QT = at.tile([P, DT, S], F16, tag="at_QT", bufs=1, name="QT")

        def OT_tile():
            return at.tile([P, DT, S], F16, tag="at_OT", bufs=2, name=_nm("OT"))

        def att_group(g, KTg, Vgg, QTt, OT, ot_list=(2,)):
            """scores -> fused exp -> AV for head group g. Scores for step
            tau are emitted before the AVs of step tau-1 so PE stays ahead
            of the scalar engine; feed() interleaves background chunks. The
            AV accumulator alternates over ot_list so the eviction chain of
            head-pair j overlaps head-pair j+1."""
            stp_i = [0]

            def av(tau, est, j, ot):
                for half in range(2):
                    nc.tensor.matmul(ot[0:65, half, :],
                                     Vgg[:, tau, 2 * j + half, :],
                                     est[:, half, :],
                                     start=(tau == 0), stop=(tau == TT - 1))

            for j in range(4):
                ot = pt[ot_list[j % len(ot_list)]]
                dtile = g * 4 + j
                prev = None
                for tau in range(TT):
                    stp = pt[stp_i[0]]
                    stp_i[0] ^= 1
                    for half in range(2):
                        r0 = 64 * half
                        nc.tensor.matmul(
                            stp[:, half, :],
                            KTg[r0:r0 + 64, j, tau * P:(tau + 1) * P],
                            QTt[r0:r0 + 64, dtile, :],
                            start=True, stop=True, tile_position=(r0, 0),
                        )
                    feed(1)
                    if prev is not None:
                        av(prev[0], prev[1], j, ot)
                    est = est_tile()
                    nc.scalar.activation(
                        est[:], stp[:], mybir.ActivationFunctionType.Exp,
                        scale=0.125)
                    prev = (tau, est)
                av(prev[0], prev[1], j, ot)
                # normalize: psum row 64 of each half is the softmax
                # denominator (V-aug ones column).
                bch = bc_tile()
                for half in range(2):
                    st = stat_tile()
                    nc.vector.tensor_copy(st[64:65, :], ot[64:65, half, :])
                    sh = stat_tile()
                    nc.scalar.dma_start(sh[0:1, :], st[64:65, :])
                    rc = stat_tile()
                    nc.vector.reciprocal_approx_fast(rc[0:1, :], sh[0:1, :])
                    nc.gpsimd.partition_broadcast(bch[:, half, :], rc[0:1, :])
                nc.vector.tensor_tensor(
                    OT[0:64, dtile, :], ot[0:64, 0, :], bch[:, 0, :], MULT)
                stg = avstg_tile()
                nc.vector.tensor_tensor(
                    stg[:], ot[0:64, 1, :], bch[:, 1, :], MULT)
                nc.sync.dma_start(OT[64:128, dtile, :], stg[:])
                feed(2)

        # ---- SA prep (PE fully busy, no dependencies) --------------------
        KT_sa = [KT_tile(), KT_tile()]
        Vg_sa = [Vg_tile(), Vg_tile()]
        streams.append(kproj_stream(w["wksa"], srcx, bias["bksa"], KT_sa[0], 0, accs=(0, 1, 2, 3)))
        drain()
        streams.append(vproj_stream(w["wvsa"], srcx, 0, Vg_sa[0], accs=(0, 1, 2, 3)))
        drain()
        proj2(QT, w["wqsa"], lambda k: xq[:, k, :], bias["bqsa"])
        streams.append(kproj_stream(w["wksa"], srcx, bias["bksa"], KT_sa[1], 1, accs=(0, 1, 2, 3)))
        drain()
        streams.append(vproj_stream(w["wvsa"], srcx, 1, Vg_sa[1], accs=(0, 1, 2, 3)))
        drain()

        # ---- SA attention; CA K/V projections ride along as background ---
        OT1 = OT_tile()
        KT_ca = [KT_tile(), KT_tile()]
        Vg_ca = [Vg_tile(), Vg_tile()]
        streams.append(kproj_stream(w["wkca"], srcm, bias["bkca"], KT_ca[0], 0))
        streams.append(vproj_stream(w["wvca"], srcm, 0, Vg_ca[0]))
        att_group(0, KT_sa[0], Vg_sa[0], QT, OT1)
        streams.append(kproj_stream(w["wkca"], srcm, bias["bkca"], KT_ca[1], 1))
        att_group(1, KT_sa[1], Vg_sa[1], QT, OT1)

        streams.append(vproj_stream(w["wvca"], srcm, 1, Vg_ca[1]))
        r1 = r_tile()
        proj2(r1, w["wosa"], lambda k: OT1[:, k, :], bias["bosa"],
              resid_fn=lambda o: xq[:, o, :], pts=(0, 1),
              per_chunk=lambda c: ln_stats(r1, pt[2], [2 * c, 2 * c + 1]))
        x1T = lnout_tile()
        layernorm(r1, bias["g1"], bias["be1"], x1T, pt[2], stats_done=True)

        # ---- CA ----------------------------------------------------------
        proj2(QT, w["wqca"], lambda k: x1T[:, k, :], bias["bqca"])
        OT2 = OT_tile()
        att_group(0, KT_ca[0], Vg_ca[0], QT, OT2, ot_list=(2, 3))
        att_group(1, KT_ca[1], Vg_ca[1], QT, OT2, ot_list=(2, 3))
        drain()

        r2 = r_tile()
        proj2(r2, w["woca"], lambda k: OT2[:, k, :], bias["boca"],
              resid_fn=lambda o: x1T[:, o, :], pts=(0, 1),
              per_chunk=lambda c: ln_stats(r2, pt[3], [2 * c, 2 * c + 1]))
        x2T = lnout_tile()
        layernorm(r2, bias["g2"], bias["be2"], x2T, pt[3], stats_done=True)

    # ======================= MLP phase ===================================
    with tc.tile_pool(name="mlp", bufs=1) as mlp:
        hT = mlp.tile([P, FF // P, S], F16, tag="hT")
        for c in range(16):
            acc = pt[c % 4]
            for half in range(2):
                slab = wslab()
                sl = slab[:].rearrange("p (kk c) -> p kk c", c=2 * P)
                nc.sync.dma_start(
                    sl[:],
                    w1[half * 4 * P:(half + 1) * 4 * P,
                       c * 2 * P:(c + 1) * 2 * P]
                    .rearrange("(kk p) c -> p kk c", p=P),
                )
                for kk in range(4):
                    k = half * 4 + kk
                    for f2 in range(2):
                        nc.tensor.matmul(
                            acc[:, f2, :], sl[:, kk, f2 * P:(f2 + 1) * P],
                            x2T[:, k, :],
                            start=(k == 0), stop=(k == DT - 1),
                        )
            for f2 in range(2):
                fi = c * 2 + f2
                nc.scalar.activation(
                    hT[:, fi, :], acc[:, f2, :],
                    mybir.ActivationFunctionType.Gelu,
                    bias=b1_sb[:, fi:fi + 1])

        r3 = r_tile()
        sum_ap = pt[3][0:1, 0, :]
        sq_ap = pt[3][0:1, 1, :]
        for c in range(4):
            acc = pt[c % 3]
            for part in range(8):
                slab = wslab()
                sl = slab[:].rearrange("p (kk c) -> p kk c", c=2 * P)
                nc.sync.dma_start(
                    sl[:],
                    w2[part * 4 * P:(part + 1) * 4 * P,
                       c * 2 * P:(c + 1) * 2 * P]
                    .rearrange("(kk p) c -> p kk c", p=P),
                )
                for kk in range(4):
                    f = part * 4 + kk
                    for o2 in range(2):
                        nc.tensor.matmul(
                            acc[:, o2, :], sl[:, kk, o2 * P:(o2 + 1) * P],
                            hT[:, f, :],
                            start=(f == 0), stop=(f == FF // P - 1),
                        )
            for o2 in range(2):
                o = c * 2 + o2
                nc.vector.scalar_tensor_tensor(
                    r3[:, o, :], acc[:, o2, :], bias["b2"][:, o:o + 1],
                    x2T[:, o, :], ADD, ADD)
                sq = sq_tile()
                nc.vector.tensor_tensor(
                    sq[:], _f32(r3[:, o, :]), _f32(r3[:, o, :]), MULT)
                nc.tensor.matmul(sum_ap, ones_col[:], r3[:, o, :],
                                 start=(o == 0), stop=(o == DT - 1))
                nc.tensor.matmul(sq_ap, ones_col[:], sq[:],
                                 start=(o == 0), stop=(o == DT - 1))

        # LN3 (stats already accumulated in pt[3]) + per-k output DMA
        mu = stat_r_tile()
        nc.vector.tensor_scalar_mul(mu[:], sum_ap, 1.0 / D)
        var = stat_tile()
        nc.vector.tensor_scalar_mul(var[0:1, :], sq_ap, 1.0 / D)
        aux = stat_tile()
        nc.vector.tensor_tensor(aux[0:1, :], _f32(mu[:]), _f32(mu[:]), MULT)
        nc.vector.scalar_tensor_tensor(var[0:1, :], var[0:1, :],
                                       eps_col[0:1, :], aux[0:1, :], ADD, SUB)
        rstd = rsqrt_row(var)
        nc.tensor.matmul(pt[3][:, 0, :], ones_row[:], mu[:],
                         start=True, stop=True)
        nc.tensor.matmul(pt[3][:, 1, :], ones_row[:], rstd[:],
                         start=True, stop=True)
        for k in range(DT):
            t1 = nrm_tile()
            nc.vector.tensor_tensor(t1[:], _f32(r3[:, k, :]), pt[3][:, 0, :], SUB)
            nc.vector.tensor_tensor(t1[:], t1[:], pt[3][:, 1, :], MULT)
            outsb = glob.tile([P, S], F32, tag="outsb", bufs=2, name=_nm("ou"))
            eng = nc.gpsimd if k % 2 else nc.vector
            eng.tensor_scalar(
                outsb[:], t1[:], bias["g3"][:, k:k + 1], bias["be3"][:, k:k + 1],
                MULT, ADD)
            nc.sync.dma_start(outT[k * P:(k + 1) * P, :], outsb[:])

    ps.release()
    wp.release()
    glob.release()


def _get_nc():
    if "nc" not in _CACHE:
        _CACHE["nc"] = build()
    return _CACHE["nc"]


def kernel(x, mem, sa_in_w, sa_in_b, sa_out_w, sa_out_b,
           ca_in_w, ca_in_b, ca_out_w, ca_out_b,
           ff_w1, ff_b1, ff_w2, ff_b2,
           ln1_g, ln1_b, ln2_g, ln2_b, ln3_g, ln3_b, n_heads=16):
    x = np.asarray(x, np.float32)
    mem = np.asarray(mem, np.float32)
    B = x.shape[0]

    def T_(a):
        return np.ascontiguousarray(np.asarray(a, np.float32).T.astype(np.float16))

    wq_sa, wk_sa, wv_sa = (np.asarray(a, np.float32) for a in np.split(np.asarray(sa_in_w), 3, axis=0))
    bq_sa, bk_sa, bv_sa = (np.asarray(a, np.float32) for a in np.split(np.asarray(sa_in_b), 3))
    wq_ca, wk_ca, wv_ca = (np.asarray(a, np.float32) for a in np.split(np.asarray(ca_in_w), 3, axis=0))
    bq_ca, bk_ca, bv_ca = (np.asarray(a, np.float32) for a in np.split(np.asarray(ca_in_b), 3))
    sa_out_w = np.asarray(sa_out_w, np.float32)
    ca_out_w = np.asarray(ca_out_w, np.float32)

    common = {
        "wqsa": T_(wq_sa), "wksa": T_(wk_sa), "wvsa": T_(wv_sa), "wosa": T_(sa_out_w),
        "wqca": T_(wq_ca), "wkca": T_(wk_ca), "wvca": T_(wv_ca), "woca": T_(ca_out_w),
        "w1": T_(ff_w1), "w2": T_(ff_w2),
        "bqsa": bq_sa, "bksa": bk_sa,
        "bosa": np.asarray(sa_out_b, np.float32) + sa_out_w @ bv_sa,
        "bqca": bq_ca, "bkca": bk_ca,
        "boca": np.asarray(ca_out_b, np.float32) + ca_out_w @ bv_ca,
        "b1": np.asarray(ff_b1, np.float32), "b2": np.asarray(ff_b2, np.float32),
        "g1": np.asarray(ln1_g, np.float32), "be1": np.asarray(ln1_b, np.float32),
        "g2": np.asarray(ln2_g, np.float32), "be2": np.asarray(ln2_b, np.float32),
        "g3": np.asarray(ln3_g, np.float32), "be3": np.asarray(ln3_b, np.float32),
    }

    in_maps = []
    for c in range(NC):
        b, h = c // 2, c % 2
        xbT = T_(x[b])
        in_maps.append({
            **common,
            "xqT": np.ascontiguousarray(xbT[:, h * S: (h + 1) * S]),
            "xkvT": xbT,
            "memT": T_(mem[b]),
        })

    nc = _get_nc()
    res = run_bass_kernel_spmd(nc, in_maps, core_ids=list(range(NC)))

    out = np.empty((B, T, D), np.float32)
    for c in range(NC):
        b, h = c // 2, c % 2
        out[b, h * S: (h + 1) * S, :] = res.results[c]["outT"].T
    return out
